# revision 1
# baseline (speedup 1.0000x reference)
"""EGNN (4-layer) Trainium2 kernel, 8 NeuronCores, edge-sharded.

Strategy:
 - Edges are sorted by destination row and assigned to the core that owns the
   row's node range (2500 nodes/core, padded to 2560 = 20 windows of 128).
 - The edge-MLP first matmul is decomposed: concat([h_r,h_c,rad,ea]) @ W1 =
   P'[row] + Q'[col] + W1e.T@eaT + wr3.T@(c_r*c_c), where P' = h@W1a + |c|^2*wr
   and Q' = h@W1b + |c|^2*wr are per-node tables recomputed each layer
   (16x less matmul work than edge-side) and gathered per edge.
 - Tables live in SBUF; gathers are bf16 transposed SBUF-source dma_gather
   (features on partitions, edges on free) so the second edge matmul uses the
   gathered m1^T directly as the stationary operand.
 - segment_sum: per 128-node window, one-hot indicator built on-device
   (is_equal vs iota) feeds an accumulating matmul producing agg^T directly.
 - Q' shards are AllGather'd across the 8 cores each layer.
 - Node MLP, residual and final LayerNorm run on device; host only sorts /
   permutes indices and concatenates the 8 output shards.
"""

import math
import os
import sys
from contextlib import ExitStack

import numpy as np

sys.path.insert(0, "/opt/trn_rl_repo")

import ml_dtypes  # noqa: E402

BF = ml_dtypes.bfloat16

N = 20000
NCORES = 8
NPC = 2500          # nodes per core
NPCP = 2560         # padded nodes per core
WINS = 20           # node windows of 128 per core
H = 128
DE = 32
L = 4
EPS = 1e-5

_CACHE = {}


def _groups(T):
    """Split T tiles-of-128 into free-dim groups of <=512."""
    out = []
    left = T
    while left > 0:
        g = min(left, 4)
        out.append(g * 128)
        left -= g
    return out


def _build(T, flags):
    """Build the SPMD Bass program (same for all cores)."""
    import concourse.bacc as bacc
    import concourse.tile as tile
    from concourse import mybir

    eb1_nz, eb2_nz, nb1_nz, nb2_nz, lnb_nz = flags
    EW = T * 128
    EPAD = WINS * EW
    GRPS = _groups(T)

    f32 = mybir.dt.float32
    bf16 = mybir.dt.bfloat16
    i16 = mybir.dt.int16
    AX = mybir.AxisListType.X
    OP = mybir.AluOpType
    AF = mybir.ActivationFunctionType

    nc = bacc.Bacc(
        "TRN2",
        target_bir_lowering=False,
        debug=False,
        enable_asserts=False,
        num_devices=NCORES,
    )

    def din(name, shape, dt):
        return nc.dram_tensor(name, list(shape), dt, kind="ExternalInput").ap()

    h_d = din("h", (NPCP, H), f32)
    coords_d = din("coords", (NPCP, 4), f32)
    eaT_d = din("eaT", (DE, EPAD), bf16)
    cr6_d = din("cr6T", (8, EPAD), f32)
    idxp_d = din("idxp", (128, EPAD // 16), i16)
    idxq_d = din("idxq", (128, EPAD // 16), i16)
    rowrel_d = din("rowrel", (128, EPAD // 128), f32)
    w1h_d = din("w1h", (L, H, H), bf16)
    w1c_d = din("w1c", (L, H, H), bf16)
    w1e_d = din("w1e", (L, DE, H), bf16)
    wr3_d = din("wr3", (L, 3, H), bf16)
    wrb_d = din("wrb", (L, H, H), f32)
    ew2_d = din("ew2", (L, H, H), bf16)
    nw1h_d = din("nw1h", (L, H, H), bf16)
    nw1a_d = din("nw1a", (L, H, H), bf16)
    nw2_d = din("nw2", (L, H, H), bf16)
    eb1_d = din("eb1T", (H, L), f32)
    nb1_d = din("nb1T", (H, L), f32)
    nb2_d = din("nb2T", (H, L), f32)
    eb2b_d = din("eb2b", (L, H, H), f32) if eb2_nz else None
    lng_d = din("lngb", (H, H), f32)
    lnb_d = din("lnbb", (H, H), f32) if lnb_nz else None
    iota_d = din("iota", (H, H), bf16)
    idb_d = din("identb", (H, H), bf16)
    idf_d = din("identf", (H, H), f32)

    out_d = nc.dram_tensor("out", [NPCP, H], f32, kind="ExternalOutput").ap()

    with ExitStack() as ctx:
        tc = ctx.enter_context(tile.TileContext(nc))
        const = ctx.enter_context(tc.tile_pool(name="const", bufs=1))
        resid = ctx.enter_context(tc.tile_pool(name="resid", bufs=1))
        dram = ctx.enter_context(tc.tile_pool(name="dram", bufs=1, space="DRAM"))
        gpool = ctx.enter_context(tc.tile_pool(name="gpool", bufs=2))
        wpool = ctx.enter_context(tc.tile_pool(name="wpool", bufs=2))
        work = ctx.enter_context(tc.tile_pool(name="work", bufs=3))
        ps512 = ctx.enter_context(tc.tile_pool(name="ps512", bufs=3, space="PSUM"))
        psm = ctx.enter_context(tc.tile_pool(name="psm", bufs=3, space="PSUM"))
        psagg = ctx.enter_context(tc.tile_pool(name="psagg", bufs=2, space="PSUM"))

        sync = nc.sync

        # ---------- constants ----------
        iota_sb = const.tile([H, H], bf16)
        sync.dma_start(out=iota_sb[:], in_=iota_d[:])
        idb_sb = const.tile([H, H], bf16)
        sync.dma_start(out=idb_sb[:], in_=idb_d[:])
        idf_sb = const.tile([H, H], f32)
        sync.dma_start(out=idf_sb[:], in_=idf_d[:])
        lng_sb = const.tile([H, H], f32)
        sync.dma_start(out=lng_sb[:], in_=lng_d[:])
        lnb_sb = None
        if lnb_nz:
            lnb_sb = const.tile([H, H], f32)
            sync.dma_start(out=lnb_sb[:], in_=lnb_d[:])
        eb1_sb = const.tile([H, L], f32)
        sync.dma_start(out=eb1_sb[:], in_=eb1_d[:])
        nb1_sb = const.tile([H, L], f32)
        sync.dma_start(out=nb1_sb[:], in_=nb1_d[:])
        nb2_sb = const.tile([H, L], f32)
        sync.dma_start(out=nb2_sb[:], in_=nb2_d[:])

        def load_w(name, d, p, dt):
            t = const.tile([p, L, H], dt, name=name)
            sync.dma_start(out=t[:], in_=d.rearrange("l k f -> k l f"))
            return t

        w1h_sb = load_w("w1h_sb", w1h_d, H, bf16)
        w1c_sb = load_w("w1c_sb", w1c_d, H, bf16)
        w1e_sb = load_w("w1e_sb", w1e_d, DE, bf16)
        wr3_sb = load_w("wr3_sb", wr3_d, 3, bf16)
        wrb_sb = load_w("wrb_sb", wrb_d, H, f32)
        ew2_sb = load_w("ew2_sb", ew2_d, H, bf16)
        nw1h_sb = load_w("nw1h_sb", nw1h_d, H, bf16)
        nw1a_sb = load_w("nw1a_sb", nw1a_d, H, bf16)
        nw2_sb = load_w("nw2_sb", nw2_d, H, bf16)
        eb2b_sb = load_w("eb2b_sb", eb2b_d, H, f32) if eb2_nz else None

        # ---------- resident state ----------
        hT = resid.tile([H, NPCP], bf16)
        zTa = resid.tile([H, NPCP], bf16)
        s_sb = resid.tile([H, WINS], f32)
        p_sb = resid.tile([H, WINS * H], bf16)
        q_sb = resid.tile([H, 160 * H], bf16)
        idxp_sb = resid.tile([128, EPAD // 16], i16)
        sync.dma_start(out=idxp_sb[:], in_=idxp_d[:])
        idxq_sb = resid.tile([128, EPAD // 16], i16)
        sync.dma_start(out=idxq_sb[:], in_=idxq_d[:])
        rowrel_sb = resid.tile([128, EPAD // 128], f32)
        sync.dma_start(out=rowrel_sb[:], in_=rowrel_d[:])

        t1_dram = dram.tile([3, EPAD], bf16)
        qown_dram = dram.tile([NPCP, H], bf16)

        # ---------- layer-0 setup: h^T, |c|^2, t1 = c_r*c_c ----------
        for j in range(WINS):
            jsl = slice(j * H, (j + 1) * H)
            h_in = work.tile([H, H], f32, tag="h_in")
            sync.dma_start(out=h_in[:], in_=h_d[jsl, :])
            pst = psm.tile([H, H], f32, tag="pm")
            nc.tensor.transpose(pst[:], h_in[:], idf_sb[:])
            nc.vector.tensor_copy(hT[:, jsl], pst[:])

            c_in = work.tile([H, 4], f32, tag="c_in")
            sync.dma_start(out=c_in[:], in_=coords_d[jsl, :])
            csq = work.tile([H, 4], f32, tag="csq")
            nc.vector.tensor_mul(csq[:], c_in[:], c_in[:])
            nc.vector.reduce_sum(s_sb[:, j : j + 1], csq[:, 0:3], AX)

        for w in range(WINS):
            wsl = slice(w * EW, (w + 1) * EW)
            crt = wpool.tile([3, EW], f32, tag="crt")
            sync.dma_start(out=crt[:], in_=cr6_d[0:3, wsl])
            cct = wpool.tile([3, EW], f32, tag="cct")
            sync.dma_start(out=cct[:], in_=cr6_d[3:6, wsl])
            t1w = wpool.tile([3, EW], bf16, tag="t1w")
            nc.vector.tensor_mul(t1w[:], crt[:], cct[:])
            sync.dma_start(out=t1_dram[:, wsl], in_=t1w[:])

        # ---------- layers ----------
        for l in range(L):
            # node tables P', Q'
            for j in range(WINS):
                jsl = slice(j * H, (j + 1) * H)
                tfold = work.tile([H, H], f32, tag="tfold")
                nc.vector.tensor_scalar_mul(
                    tfold[:], wrb_sb[:, l, :], s_sb[:, j : j + 1]
                )
                psp = psm.tile([H, H], f32, tag="pm")
                nc.tensor.matmul(
                    psp[:], hT[:, jsl], w1h_sb[:, l, :], start=True, stop=True
                )
                nc.vector.tensor_tensor(p_sb[:, jsl], psp[:], tfold[:], OP.add)
                psq = psm.tile([H, H], f32, tag="pm")
                nc.tensor.matmul(
                    psq[:], hT[:, jsl], w1c_sb[:, l, :], start=True, stop=True
                )
                qt = work.tile([H, H], bf16, tag="qt")
                nc.vector.tensor_tensor(qt[:], psq[:], tfold[:], OP.add)
                sync.dma_start(out=qown_dram[jsl, :], in_=qt[:])

            qfull_dram = dram.tile(
                [NCORES * NPCP, H], bf16, addr_space="Shared", name=f"qfull_{l}"
            )
            nc.gpsimd.collective_compute(
                "AllGather",
                mybir.AluOpType.bypass,
                replica_groups=[list(range(NCORES))],
                ins=[qown_dram.opt()],
                outs=[qfull_dram.opt()],
            )
            sync.dma_start(
                out=q_sb[:], in_=qfull_dram.rearrange("(p r) f -> p (r f)", p=128)
            )

            # edge pass
            for w in range(WINS):
                wsl = slice(w * EW, (w + 1) * EW)
                isl = slice(w * (EW // 16), (w + 1) * (EW // 16))
                gp = gpool.tile([128, 1, EW], bf16, tag="gp")
                nc.gpsimd.dma_gather(
                    gp[:],
                    p_sb[:],
                    idxp_sb[:, isl],
                    EW,
                    EW,
                    H,
                    transpose=True,
                    single_packet=False,
                    sbuf_tokens_per_rank=128,
                    sbuf_free_dim_per_rank=256,
                )
                gq = gpool.tile([128, 1, EW], bf16, tag="gq")
                nc.gpsimd.dma_gather(
                    gq[:],
                    q_sb[:],
                    idxq_sb[:, isl],
                    EW,
                    EW,
                    H,
                    transpose=True,
                    single_packet=False,
                    sbuf_tokens_per_rank=128,
                    sbuf_free_dim_per_rank=256,
                )
                eaw = wpool.tile([DE, EW], bf16, tag="eaw")
                sync.dma_start(out=eaw[:], in_=eaT_d[:, wsl])
                t1r = wpool.tile([3, EW], bf16, tag="t1r")
                sync.dma_start(out=t1r[:], in_=t1_dram[:, wsl])

                pagg = psagg.tile([H, H], f32, tag="pagg")
                gtile = 0
                off = 0
                for gsz in GRPS:
                    gsl = slice(off, off + gsz)
                    ps1 = ps512.tile([H, 512], f32, tag="big")
                    nc.tensor.matmul(
                        ps1[:, :gsz], w1e_sb[:, l, :], eaw[:, gsl],
                        start=True, stop=False,
                    )
                    nc.tensor.matmul(
                        ps1[:, :gsz], wr3_sb[:, l, :], t1r[:, gsl],
                        start=False, stop=True,
                    )
                    gsum = work.tile([H, 512], bf16, tag="gsum")
                    nc.vector.tensor_tensor(
                        gsum[:, :gsz], gp[:, 0, gsl], gq[:, 0, gsl], OP.add
                    )
                    m1t = work.tile([H, 512], bf16, tag="m1t")
                    pre1 = work.tile([H, 512], f32, tag="pre1")
                    nc.vector.tensor_tensor(
                        pre1[:, :gsz], ps1[:, :gsz], gsum[:, :gsz], OP.add
                    )
                    bias1 = eb1_sb[:, l : l + 1] if eb1_nz else 0.0
                    nc.scalar.activation(m1t[:, :gsz], pre1[:, :gsz], AF.Silu, bias=bias1)

                    for t in range(gsz // 128):
                        tsl = slice(t * 128, (t + 1) * 128)
                        gti = w * T + gtile
                        pm2 = psm.tile([H, H], f32, tag="pm")
                        nc.tensor.matmul(
                            pm2[:], m1t[:, tsl], ew2_sb[:, l, :],
                            start=True, stop=True,
                        )
                        m2s = work.tile([H, H], bf16, tag="m2s")
                        if eb2_nz:
                            tm2 = work.tile([H, H], f32, tag="tm2")
                            nc.vector.tensor_tensor(
                                tm2[:], pm2[:], eb2b_sb[:, l, :], OP.add
                            )
                            nc.scalar.activation(m2s[:], tm2[:], AF.Silu)
                        else:
                            nc.scalar.activation(m2s[:], pm2[:], AF.Silu)
                        ind = work.tile([H, H], bf16, tag="ind")
                        nc.vector.tensor_scalar(
                            ind[:], iota_sb[:],
                            rowrel_sb[:, gti : gti + 1], None, OP.is_equal,
                        )
                        nc.tensor.matmul(
                            pagg[:], m2s[:], ind[:],
                            start=(gtile == 0), stop=(gtile == T - 1),
                        )
                        gtile += 1
                    off += gsz
                nc.vector.tensor_copy(zTa[:, w * H : (w + 1) * H], pagg[:])

            # node MLP (+ residual)
            for g5 in range(NPCP // 512):
                sl = slice(g5 * 512, (g5 + 1) * 512)
                psu = ps512.tile([H, 512], f32, tag="big")
                nc.tensor.matmul(
                    psu[:], nw1h_sb[:, l, :], hT[:, sl], start=True, stop=False
                )
                nc.tensor.matmul(
                    psu[:], nw1a_sb[:, l, :], zTa[:, sl], start=False, stop=True
                )
                u = work.tile([H, 512], bf16, tag="u")
                biasn = nb1_sb[:, l : l + 1] if nb1_nz else 0.0
                nc.scalar.activation(u[:], psu[:], AF.Silu, bias=biasn)
                pso = ps512.tile([H, 512], f32, tag="big")
                nc.tensor.matmul(pso[:], nw2_sb[:, l, :], u[:], start=True, stop=True)
                if l == 0:
                    if nb2_nz:
                        nc.vector.tensor_scalar_add(
                            hT[:, sl], pso[:], nb2_sb[:, l : l + 1]
                        )
                    else:
                        nc.vector.tensor_copy(hT[:, sl], pso[:])
                else:
                    if nb2_nz:
                        nc.vector.scalar_tensor_tensor(
                            hT[:, sl], pso[:], nb2_sb[:, l : l + 1], hT[:, sl],
                            OP.add, OP.add,
                        )
                    else:
                        nc.vector.tensor_tensor(hT[:, sl], pso[:], hT[:, sl], OP.add)

        # ---------- LayerNorm + output ----------
        inv = 1.0 / H
        for j in range(WINS):
            jsl = slice(j * H, (j + 1) * H)
            pst = psm.tile([H, H], bf16, tag="pm")
            nc.tensor.transpose(pst[:], hT[:, jsl], idb_sb[:])
            hn = work.tile([H, H], f32, tag="hn")
            nc.vector.tensor_copy(hn[:], pst[:])
            mu = work.tile([H, 1], f32, tag="mu")
            nc.vector.reduce_sum(mu[:], hn[:], AX)
            nc.vector.tensor_scalar_mul(mu[:], mu[:], inv)
            xc = work.tile([H, H], f32, tag="xc")
            nc.vector.tensor_scalar_sub(xc[:], hn[:], mu[:])
            sq = work.tile([H, H], f32, tag="sq")
            nc.vector.tensor_mul(sq[:], xc[:], xc[:])
            var = work.tile([H, 1], f32, tag="var")
            nc.vector.reduce_sum(var[:], sq[:], AX)
            sd = work.tile([H, 1], f32, tag="sd")
            nc.vector.tensor_scalar(sd[:], var[:], inv, EPS, OP.mult, OP.add)
            nc.scalar.activation(sd[:], sd[:], mybir.ActivationFunctionType.Sqrt)
            rstd = work.tile([H, 1], f32, tag="rstd")
            nc.vector.reciprocal(rstd[:], sd[:])
            on = work.tile([H, H], f32, tag="on")
            nc.vector.tensor_scalar_mul(on[:], xc[:], rstd[:])
            nc.vector.tensor_mul(on[:], on[:], lng_sb[:])
            if lnb_nz:
                nc.vector.tensor_add(on[:], on[:], lnb_sb[:])
            sync.dma_start(out=out_d[jsl, :], in_=on[:])

    nc.compile()
    return nc


def _wrap_idx(v):
    """idx i -> [i%16 partition, i//16 free], replicated to 128 partitions."""
    n = v.shape[0]
    t = v.reshape(n // 16, 16).T.astype(np.int16)
    return np.tile(t, (8, 1))


def kernel(**inputs):
    from concourse.bass_utils import run_bass_kernel_spmd

    h = np.asarray(inputs["h"], np.float32)
    coords = np.asarray(inputs["coords"], np.float32)
    edge_attr = np.asarray(inputs["edge_attr"], np.float32)
    edges = np.asarray(inputs["edges"]).astype(np.int64)
    ew1 = np.asarray(inputs["edge_w1"], np.float32)
    eb1 = np.asarray(inputs["edge_b1"], np.float32)
    ew2 = np.asarray(inputs["edge_w2"], np.float32)
    eb2 = np.asarray(inputs["edge_b2"], np.float32)
    nw1 = np.asarray(inputs["node_w1"], np.float32)
    nb1 = np.asarray(inputs["node_b1"], np.float32)
    nw2 = np.asarray(inputs["node_w2"], np.float32)
    nb2 = np.asarray(inputs["node_b2"], np.float32)
    ln_g = np.asarray(inputs["ln_g"], np.float32)
    ln_b = np.asarray(inputs["ln_b"], np.float32)

    E = edges.shape[1]
    row, col = edges[0], edges[1]

    # ---- sort edges into (core, window) slots, uniform budget T ----
    owner = row // NPC
    gw = owner * WINS + (row - owner * NPC) // 128
    order = np.argsort(gw, kind="stable")
    counts = np.bincount(gw, minlength=NCORES * WINS)
    T = int(math.ceil(counts.max() / 128))
    EW = T * 128
    EPAD = WINS * EW

    gws = gw[order]
    starts = np.zeros(NCORES * WINS, np.int64)
    starts[1:] = np.cumsum(counts)[:-1]
    pos = np.arange(E) - starts[gws]
    slot = (gws % WINS) * EW + pos
    core = gws // WINS

    rowS = np.zeros((NCORES, EPAD), np.int64)
    colS = np.zeros((NCORES, EPAD), np.int64)
    valid = np.zeros((NCORES, EPAD), bool)
    eaS = np.zeros((NCORES, EPAD, DE), np.float32)
    rowS[core, slot] = row[order]
    colS[core, slot] = col[order]
    valid[core, slot] = True
    eaS[core, slot] = edge_attr[order]

    karr = np.arange(NCORES)[:, None]
    wloc = (np.arange(EPAD) // EW)[None, :]
    rowrel = np.where(valid, rowS - karr * NPC - 128 * wloc, -1).astype(np.float32)
    idxp = np.where(valid, rowS - karr * NPC, 0).astype(np.int64)
    gcol = (colS // NPC) * NPCP + colS % NPC
    idxq = np.where(valid, (gcol % 160) * 128 + gcol // 160, 0).astype(np.int64)

    crS = np.where(valid[..., None], coords[rowS], 0.0).astype(np.float32)
    ccS = np.where(valid[..., None], coords[colS], 0.0).astype(np.float32)

    # ---- weights ----
    w1h = ew1[:, 0:H, :]
    w1c = ew1[:, H : 2 * H, :]
    wr = ew1[:, 2 * H, :]          # [L, H]
    w1e = ew1[:, 2 * H + 1 :, :]   # [L, DE, H]
    wr3 = np.repeat((-2.0 * wr)[:, None, :], 3, axis=1)
    wrb = np.repeat(wr[:, None, :], H, axis=1).astype(np.float32)
    nw1h = nw1[:, :H, :]
    nw1a = nw1[:, H:, :]

    flags = (
        bool(np.any(eb1)), bool(np.any(eb2)),
        bool(np.any(nb1)), bool(np.any(nb2)), bool(np.any(ln_b)),
    )

    key = (T, flags)
    if key not in _CACHE:
        _CACHE[key] = _build(T, flags)
    nc = _CACHE[key]

    iota = np.tile(np.arange(H, dtype=np.float32), (H, 1))
    ident = np.eye(H, dtype=np.float32)

    shared = {
        "w1h": w1h.astype(BF), "w1c": w1c.astype(BF), "w1e": w1e.astype(BF),
        "wr3": wr3.astype(BF), "wrb": wrb, "ew2": ew2.astype(BF),
        "nw1h": nw1h.astype(BF), "nw1a": nw1a.astype(BF), "nw2": nw2.astype(BF),
        "eb1T": np.ascontiguousarray(eb1.T), "nb1T": np.ascontiguousarray(nb1.T),
        "nb2T": np.ascontiguousarray(nb2.T),
        "lngb": np.tile(ln_g, (H, 1)).astype(np.float32),
        "iota": iota.astype(BF), "identb": ident.astype(BF), "identf": ident,
    }
    if flags[1]:
        shared["eb2b"] = np.repeat(eb2[:, None, :], H, axis=1).astype(np.float32)
    if flags[4]:
        shared["lnbb"] = np.tile(ln_b, (H, 1)).astype(np.float32)

    in_maps = []
    for k in range(NCORES):
        hk = np.zeros((NPCP, H), np.float32)
        hk[:NPC] = h[k * NPC : (k + 1) * NPC]
        ck = np.zeros((NPCP, 4), np.float32)
        ck[:NPC, :3] = coords[k * NPC : (k + 1) * NPC]
        cr6 = np.zeros((8, EPAD), np.float32)
        cr6[0:3] = crS[k].T
        cr6[3:6] = ccS[k].T
        m = {
            "h": hk,
            "coords": ck,
            "eaT": np.ascontiguousarray(eaS[k].T).astype(BF),
            "cr6T": cr6,
            "idxp": _wrap_idx(idxp[k]),
            "idxq": _wrap_idx(idxq[k]),
            "rowrel": np.ascontiguousarray(
                rowrel[k].reshape(EPAD // 128, 128).T
            ),
        }
        m.update(shared)
        in_maps.append(m)

    trace = bool(os.environ.get("EGNN_TRACE"))
    kw = {}
    if trace:
        kw = {"trace": True, "tmpdir": os.environ.get("EGNN_TRACE_DIR") or None}
    res = run_bass_kernel_spmd(nc, in_maps, list(range(NCORES)), **kw)
    if trace:
        print(f"HW exec time: {res.exec_time_ns} ns")
    outs = [res.results[k]["out"][:NPC] for k in range(NCORES)]
    return np.concatenate(outs, axis=0).astype(np.float32)



# revision 6
# speedup vs baseline: 1.5351x; 1.5351x over previous
"""EGNN (4-layer) Trainium2 kernel, 8 NeuronCores, edge-sharded.

Strategy (v2 — SWDGE-minimal):
 - Edges sorted by destination row; each core owns 2500 nodes (20 windows
   of 128). Per (core,window) uniform tile budget T.
 - Edge-MLP first matmul decomposed: concat([h_r,h_c,rad,ea]) @ W1 =
   P'win one-hot-matmul + Q'[col] gather + [W1e;wr3].T @ [ea;c_r*c_c],
   with P' = h@W1a + |c|^2*wr per-node tables (row side is window-local).
 - One-hot matrices ind [edge,node] (aggregation stationary) and
   indT [node,edge] (P-term moving operand) are HOST-built bf16 DRAM
   tensors streamed per window — no on-device is_equal chains, and the
   row-side "gather" is a matmul, eliminating one of two SWDGE gathers.
 - Q side: per-layer AllGather of per-core Q' tables to a shared DRAM
   buffer; one HBM-source transposed dma_gather per window (the only
   SWDGE user left).
 - Edge-input term: single K=35 matmul vs host-fused et=[ea;c_r*c_c].
 - m2 activations batched per 512-group; segment-sum via accumulating
   one-hot matmul producing agg^T directly.
"""

import math
import os
import sys
from contextlib import ExitStack

import numpy as np

sys.path.insert(0, "/opt/trn_rl_repo")

import ml_dtypes  # noqa: E402

BF = ml_dtypes.bfloat16

N = 20000
NCORES = 8
NPC = 2500          # nodes per core
NPCP = 2560         # padded nodes per core
WINS = 20           # node windows of 128 per core
NG = NCORES * NPCP  # global padded nodes
H = 128
DE = 32
KE = DE + 3         # fused edge-input contraction dim
L = 4
EPS = 1e-5

GATHER_PREP = os.environ.get("EGNN_PREP", "0") == "1"  # prepare_only + trigger_dma

_CACHE = {}


def _groups(T):
    """Split T tiles-of-128 into free-dim groups of <=512."""
    out = []
    left = T
    while left > 0:
        g = min(left, 4)
        out.append(g * 128)
        left -= g
    return out


def _build(T, flags):
    """Build the SPMD Bass program (same for all cores)."""
    import concourse.bacc as bacc
    import concourse.tile as tile
    from concourse import mybir

    eb1_nz, eb2_nz, nb1_nz, nb2_nz, lnb_nz = flags
    EW = T * 128
    EPAD = WINS * EW
    GRPS = _groups(T)

    f32 = mybir.dt.float32
    bf16 = mybir.dt.bfloat16
    i16 = mybir.dt.int16
    AX = mybir.AxisListType.X
    OP = mybir.AluOpType
    AF = mybir.ActivationFunctionType

    nc = bacc.Bacc(
        "TRN2",
        target_bir_lowering=False,
        debug=False,
        enable_asserts=False,
        num_devices=NCORES,
    )

    def din(name, shape, dt):
        return nc.dram_tensor(name, list(shape), dt, kind="ExternalInput").ap()

    hT0_d = din("hT0", (H, NPCP), bf16)
    s2_d = din("s2", (128, WINS), f32)
    idxq_d = din("idxq", (128, EPAD // 16), i16)
    et_d = din("et", (KE, EPAD), bf16)
    ind_d = din("indb", (128, EPAD), bf16)
    indT_d = din("indTb", (128, EPAD), bf16)
    w1h_d = din("w1h", (L, H, H), bf16)
    w1c_d = din("w1c", (L, H, H), bf16)
    wrb_d = din("wrb", (L, H, H), f32)
    w1et_d = din("w1et", (L, KE, H), bf16)
    ew2_d = din("ew2", (L, H, H), bf16)
    nw1h_d = din("nw1h", (L, H, H), bf16)
    nw1a_d = din("nw1a", (L, H, H), bf16)
    nw2_d = din("nw2", (L, H, H), bf16)
    eb1_d = din("eb1T", (H, L), f32) if eb1_nz else None
    nb1_d = din("nb1T", (H, L), f32) if nb1_nz else None
    nb2_d = din("nb2T", (H, L), f32) if nb2_nz else None
    eb2b_d = din("eb2b", (L, H, H), f32) if eb2_nz else None
    lng_d = din("lngb", (H, H), f32)
    lnb_d = din("lnbb", (H, H), f32) if lnb_nz else None
    idb_d = din("identb", (H, H), bf16)

    out_d = nc.dram_tensor("out", [NPCP, H], f32, kind="ExternalOutput").ap()

    with ExitStack() as ctx:
        tc = ctx.enter_context(tile.TileContext(nc))
        const = ctx.enter_context(tc.tile_pool(name="const", bufs=1))
        resid = ctx.enter_context(tc.tile_pool(name="resid", bufs=1))
        dram = ctx.enter_context(tc.tile_pool(name="dram", bufs=1, space="DRAM"))
        gpool = ctx.enter_context(tc.tile_pool(name="gpool", bufs=2))
        wpool = ctx.enter_context(tc.tile_pool(name="wpool", bufs=2))
        work = ctx.enter_context(tc.tile_pool(name="work", bufs=3))
        ps512 = ctx.enter_context(tc.tile_pool(name="ps512", bufs=3, space="PSUM"))
        psm = ctx.enter_context(tc.tile_pool(name="psm", bufs=2, space="PSUM"))
        psagg = ctx.enter_context(tc.tile_pool(name="psagg", bufs=2, space="PSUM"))

        sync = nc.sync

        # ---------- constants ----------
        idb_sb = const.tile([H, H], bf16)
        sync.dma_start(out=idb_sb[:], in_=idb_d[:])
        lng_sb = const.tile([H, H], f32)
        sync.dma_start(out=lng_sb[:], in_=lng_d[:])
        lnb_sb = None
        if lnb_nz:
            lnb_sb = const.tile([H, H], f32)
            sync.dma_start(out=lnb_sb[:], in_=lnb_d[:])
        eb1_sb = None
        if eb1_nz:
            eb1_sb = const.tile([H, L], f32)
            sync.dma_start(out=eb1_sb[:], in_=eb1_d[:])
        nb1_sb = None
        if nb1_nz:
            nb1_sb = const.tile([H, L], f32)
            sync.dma_start(out=nb1_sb[:], in_=nb1_d[:])
        nb2_sb = None
        if nb2_nz:
            nb2_sb = const.tile([H, L], f32)
            sync.dma_start(out=nb2_sb[:], in_=nb2_d[:])

        def load_w(name, d, p, dt):
            t = const.tile([p, L, H], dt, name=name)
            sync.dma_start(out=t[:], in_=d.rearrange("l k f -> k l f"))
            return t

        w1h_sb = load_w("w1h_sb", w1h_d, H, bf16)
        w1c_sb = load_w("w1c_sb", w1c_d, H, bf16)
        wrb_sb = load_w("wrb_sb", wrb_d, H, f32)
        w1et_sb = load_w("w1et_sb", w1et_d, KE, bf16)
        ew2_sb = load_w("ew2_sb", ew2_d, H, bf16)
        nw1h_sb = load_w("nw1h_sb", nw1h_d, H, bf16)
        nw1a_sb = load_w("nw1a_sb", nw1a_d, H, bf16)
        nw2_sb = load_w("nw2_sb", nw2_d, H, bf16)
        eb2b_sb = load_w("eb2b_sb", eb2b_d, H, f32) if eb2_nz else None

        # ---------- resident state ----------
        hT = resid.tile([H, NPCP], bf16)
        sync.dma_start(out=hT[:], in_=hT0_d[:])
        zTa = resid.tile([H, NPCP], bf16)
        s2_sb = resid.tile([128, WINS], f32)
        sync.dma_start(out=s2_sb[:], in_=s2_d[:])
        p_sb = resid.tile([128, WINS * H], bf16)
        idxq_sb = resid.tile([128, EPAD // 16], i16)
        sync.dma_start(out=idxq_sb[:], in_=idxq_d[:])

        qown_dram = dram.tile([NPCP, H], bf16)
        gsem = nc.alloc_semaphore("gq_dma") if GATHER_PREP else None

        # ---------- layers ----------
        for l in range(L):
            # node tables Q' then P' (Q first so the AllGather launches early)
            for j in range(WINS):
                jsl = slice(j * H, (j + 1) * H)
                tfold = work.tile([128, H], f32, tag="tfold")
                nc.vector.tensor_scalar_mul(
                    tfold[:], wrb_sb[:, l, :], s2_sb[:, j : j + 1]
                )
                psq = psm.tile([H, H], f32, tag="pm")
                nc.tensor.matmul(
                    psq[:], hT[:, jsl], w1c_sb[:, l, :], start=True, stop=True
                )
                qt = work.tile([128, H], bf16, tag="qt")
                nc.vector.tensor_tensor(qt[:], psq[:], tfold[:], OP.add)
                sync.dma_start(out=qown_dram[jsl, :], in_=qt[:])
                psp = psm.tile([H, H], f32, tag="pm")
                nc.tensor.matmul(
                    psp[:], hT[:, jsl], w1h_sb[:, l, :], start=True, stop=True
                )
                nc.vector.tensor_tensor(p_sb[:, jsl], psp[:], tfold[:], OP.add)

            qfull_dram = dram.tile(
                [NG, H], bf16, addr_space="Shared", name=f"qfull_{l}"
            )
            nc.gpsimd.collective_compute(
                "AllGather",
                mybir.AluOpType.bypass,
                replica_groups=[list(range(NCORES))],
                ins=[qown_dram.opt()],
                outs=[qfull_dram.opt()],
            )

            # edge pass
            for w in range(WINS):
                wsl = slice(w * EW, (w + 1) * EW)
                isl = slice(w * (EW // 16), (w + 1) * (EW // 16))
                psl = slice(w * H, (w + 1) * H)
                gq = gpool.tile([128, 1, EW], bf16, tag="gq")
                if GATHER_PREP:
                    nc.gpsimd.dma_gather(
                        gq[:], qfull_dram[:], idxq_sb[:, isl], EW, EW, H,
                        transpose=True, single_packet=False,
                        prepare_only=True, sem=gsem,
                    )
                    nc.gpsimd.trigger_dma(count=None)
                else:
                    nc.gpsimd.dma_gather(
                        gq[:], qfull_dram[:], idxq_sb[:, isl], EW, EW, H,
                        transpose=True, single_packet=False,
                    )
                etw = wpool.tile([KE, EW], bf16, tag="etw")
                sync.dma_start(out=etw[:], in_=et_d[:, wsl])
                indw = wpool.tile([128, EW], bf16, tag="indw")
                sync.dma_start(out=indw[:], in_=ind_d[:, wsl])
                indTw = wpool.tile([128, EW], bf16, tag="indTw")
                sync.dma_start(out=indTw[:], in_=indT_d[:, wsl])

                pagg = psagg.tile([H, H], f32, tag="pagg")
                gtile = 0
                off = 0
                for gsz in GRPS:
                    gsl = slice(off, off + gsz)
                    ps1 = ps512.tile([H, 512], f32, tag="big")
                    nc.tensor.matmul(
                        ps1[:, :gsz], w1et_sb[:, l, :], etw[:, gsl],
                        start=True, stop=False,
                    )
                    nc.tensor.matmul(
                        ps1[:, :gsz], p_sb[:, psl], indTw[:, gsl],
                        start=False, stop=True,
                    )
                    pre1 = work.tile([H, 512], f32, tag="pre1")
                    nc.vector.tensor_tensor(
                        pre1[:, :gsz], ps1[:, :gsz], gq[:, 0, gsl], OP.add
                    )
                    m1t = work.tile([H, 512], bf16, tag="m1t")
                    bias1 = eb1_sb[:, l : l + 1] if eb1_nz else 0.0
                    nc.scalar.activation(m1t[:, :gsz], pre1[:, :gsz], AF.Silu, bias=bias1)

                    pm2 = ps512.tile([H, 512], f32, tag="big")
                    for t in range(gsz // 128):
                        tsl = slice(t * 128, (t + 1) * 128)
                        nc.tensor.matmul(
                            pm2[:, tsl], m1t[:, tsl], ew2_sb[:, l, :],
                            start=True, stop=True,
                        )
                    m2s = work.tile([H, 512], bf16, tag="m2s")
                    if eb2_nz:
                        tm2 = work.tile([H, 512], f32, tag="tm2")
                        eb2g = eb2b_sb[:, l, :]
                        for t in range(gsz // 128):
                            tsl = slice(t * 128, (t + 1) * 128)
                            nc.vector.tensor_tensor(
                                tm2[:, tsl], pm2[:, tsl], eb2g, OP.add
                            )
                        nc.scalar.activation(m2s[:, :gsz], tm2[:, :gsz], AF.Silu)
                    else:
                        nc.scalar.activation(m2s[:, :gsz], pm2[:, :gsz], AF.Silu)
                    for t in range(gsz // 128):
                        tsl = slice(t * 128, (t + 1) * 128)
                        nc.tensor.matmul(
                            pagg[:], m2s[:, tsl], indw[:, gtile * 128 : gtile * 128 + 128],
                            start=(gtile == 0), stop=(gtile == T - 1),
                        )
                        gtile += 1
                    off += gsz
                nc.vector.tensor_copy(zTa[:, psl], pagg[:])

            # node MLP (+ residual)
            for g5 in range(NPCP // 512):
                sl = slice(g5 * 512, (g5 + 1) * 512)
                psu = ps512.tile([H, 512], f32, tag="big")
                nc.tensor.matmul(
                    psu[:], nw1h_sb[:, l, :], hT[:, sl], start=True, stop=False
                )
                nc.tensor.matmul(
                    psu[:], nw1a_sb[:, l, :], zTa[:, sl], start=False, stop=True
                )
                u = work.tile([H, 512], bf16, tag="u")
                biasn = nb1_sb[:, l : l + 1] if nb1_nz else 0.0
                nc.scalar.activation(u[:], psu[:], AF.Silu, bias=biasn)
                pso = ps512.tile([H, 512], f32, tag="big")
                nc.tensor.matmul(pso[:], nw2_sb[:, l, :], u[:], start=True, stop=True)
                if l == 0:
                    if nb2_nz:
                        nc.vector.tensor_scalar_add(
                            hT[:, sl], pso[:], nb2_sb[:, l : l + 1]
                        )
                    else:
                        nc.vector.tensor_copy(hT[:, sl], pso[:])
                else:
                    if nb2_nz:
                        nc.vector.scalar_tensor_tensor(
                            hT[:, sl], pso[:], nb2_sb[:, l : l + 1], hT[:, sl],
                            OP.add, OP.add,
                        )
                    else:
                        nc.vector.tensor_tensor(hT[:, sl], pso[:], hT[:, sl], OP.add)

        # ---------- LayerNorm + output ----------
        inv = 1.0 / H
        for j in range(WINS):
            jsl = slice(j * H, (j + 1) * H)
            pst = psm.tile([H, H], bf16, tag="pm")
            nc.tensor.transpose(pst[:], hT[:, jsl], idb_sb[:])
            hn = work.tile([H, H], f32, tag="hn")
            nc.vector.tensor_copy(hn[:], pst[:])
            mu = work.tile([H, 1], f32, tag="mu")
            nc.vector.reduce_sum(mu[:], hn[:], AX)
            nc.vector.tensor_scalar_mul(mu[:], mu[:], inv)
            xc = work.tile([H, H], f32, tag="xc")
            nc.vector.tensor_scalar_sub(xc[:], hn[:], mu[:])
            sq = work.tile([H, H], f32, tag="sq")
            nc.vector.tensor_mul(sq[:], xc[:], xc[:])
            var = work.tile([H, 1], f32, tag="var")
            nc.vector.reduce_sum(var[:], sq[:], AX)
            sd = work.tile([H, 1], f32, tag="sd")
            nc.vector.tensor_scalar(sd[:], var[:], inv, EPS, OP.mult, OP.add)
            nc.scalar.activation(sd[:], sd[:], mybir.ActivationFunctionType.Sqrt)
            rstd = work.tile([H, 1], f32, tag="rstd")
            nc.vector.reciprocal(rstd[:], sd[:])
            on = work.tile([H, H], f32, tag="on")
            nc.vector.tensor_scalar_mul(on[:], xc[:], rstd[:])
            nc.vector.tensor_mul(on[:], on[:], lng_sb[:])
            if lnb_nz:
                nc.vector.tensor_add(on[:], on[:], lnb_sb[:])
            sync.dma_start(out=out_d[jsl, :], in_=on[:])

    nc.compile()
    return nc


def _wrap_idx(v):
    """idx i -> [i%16 partition, i//16 free], replicated to 128 partitions."""
    n = v.shape[0]
    t = v.reshape(n // 16, 16).T.astype(np.int16)
    return np.tile(t, (8, 1))


def _prep(edges, edge_attr, coords):
    """Edge sorting + balanced node->window binning + per-edge layouts.

    Returns dict with T/EW/EPAD, per-core edge arrays (valid, eaS, t1,
    rowrel, idxq) and the node permutation perm [NC, NPCP] (node id per
    (window,slot); -1 for pads).
    """
    E = edges.shape[1]
    row, col = edges[0], edges[1]

    # balanced binning: per core, assign nodes to 20 windows (<=128 each)
    # minimizing the max per-window edge count (LPT greedy on degree)
    deg = np.bincount(row, minlength=N)
    win_of = np.zeros(N, np.int32)
    binpos = np.zeros(N, np.int32)   # w*128 + slot within core
    perm = np.full((NCORES, NPCP), -1, np.int64)
    for k in range(NCORES):
        dk = deg[k * NPC : (k + 1) * NPC]
        orderk = np.argsort(-dk, kind="stable")
        loads = np.zeros(WINS, np.int64)
        slots = np.zeros(WINS, np.int64)
        big = 1 << 60
        for nl in orderk:
            w = int(np.argmin(np.where(slots < 128, loads, big)))
            g = k * NPC + nl
            win_of[g] = w
            binpos[g] = w * 128 + slots[w]
            perm[k, w * 128 + slots[w]] = g
            slots[w] += 1
            loads[w] += dk[nl]

    owner = row // NPC
    gw = owner * WINS + win_of[row]
    order = np.argsort(gw, kind="stable")
    counts = np.bincount(gw, minlength=NCORES * WINS)
    T = int(math.ceil(counts.max() / 128))
    EW = T * 128
    EPAD = WINS * EW

    gws = gw[order]
    starts = np.zeros(NCORES * WINS, np.int64)
    starts[1:] = np.cumsum(counts)[:-1]
    pos = np.arange(E) - starts[gws]
    slot = (gws % WINS) * EW + pos
    core = gws // WINS

    rowS = np.zeros((NCORES, EPAD), np.int64)
    colS = np.zeros((NCORES, EPAD), np.int64)
    valid = np.zeros((NCORES, EPAD), bool)
    eaS = np.zeros((NCORES, EPAD, DE), np.float32)
    rowS[core, slot] = row[order]
    colS[core, slot] = col[order]
    valid[core, slot] = True
    eaS[core, slot] = edge_attr[order]

    rowrel = np.where(valid, binpos[rowS] % 128, -1).astype(np.int32)
    gcol = (colS // NPC) * NPCP + binpos[colS]
    idxq = np.where(valid, gcol, 0).astype(np.int64)

    crS = np.where(valid[..., None], coords[rowS], 0.0).astype(np.float32)
    ccS = np.where(valid[..., None], coords[colS], 0.0).astype(np.float32)
    t1 = crS * ccS  # [NC, EPAD, 3]

    return dict(T=T, EW=EW, EPAD=EPAD, valid=valid, eaS=eaS, t1=t1,
                rowrel=rowrel, idxq=idxq, perm=perm)


def kernel(**inputs):
    from concourse.bass_utils import run_bass_kernel_spmd

    h = np.asarray(inputs["h"], np.float32)
    coords = np.asarray(inputs["coords"], np.float32)
    edge_attr = np.asarray(inputs["edge_attr"], np.float32)
    edges = np.asarray(inputs["edges"]).astype(np.int64)
    ew1 = np.asarray(inputs["edge_w1"], np.float32)
    eb1 = np.asarray(inputs["edge_b1"], np.float32)
    ew2 = np.asarray(inputs["edge_w2"], np.float32)
    eb2 = np.asarray(inputs["edge_b2"], np.float32)
    nw1 = np.asarray(inputs["node_w1"], np.float32)
    nb1 = np.asarray(inputs["node_b1"], np.float32)
    nw2 = np.asarray(inputs["node_w2"], np.float32)
    nb2 = np.asarray(inputs["node_b2"], np.float32)
    ln_g = np.asarray(inputs["ln_g"], np.float32)
    ln_b = np.asarray(inputs["ln_b"], np.float32)

    P = _prep(edges, edge_attr, coords)
    T, EW, EPAD = P["T"], P["EW"], P["EPAD"]
    valid, eaS, t1 = P["valid"], P["eaS"], P["t1"]
    rowrel, idxq, perm = P["rowrel"], P["idxq"], P["perm"]

    # host-built one-hots (bf16)
    pp = np.arange(128)
    rr3 = rowrel.reshape(NCORES, EPAD // 128, 128)
    # ind[k, p, gti*128+f] = (rowrel[k, gti*128+p] == f)  [agg stationary]
    indb = (
        (rr3[:, :, :, None] == pp[None, None, None, :])
        .transpose(0, 2, 1, 3)
        .reshape(NCORES, 128, EPAD)
        .astype(BF)
    )
    # indT[k, p, e] = (rowrel[k, e] == p)                  [P-term moving]
    indTb = (
        (rr3[:, :, :, None] == pp[None, None, None, :])
        .transpose(0, 3, 1, 2)
        .reshape(NCORES, 128, EPAD)
        .astype(BF)
    )

    # ---- weights ----
    w1h = ew1[:, 0:H, :]
    w1c = ew1[:, H : 2 * H, :]
    wr = ew1[:, 2 * H, :]          # [L, H]
    w1e = ew1[:, 2 * H + 1 :, :]   # [L, DE, H]
    wr3 = np.repeat((-2.0 * wr)[:, None, :], 3, axis=1)
    w1et = np.concatenate([w1e, wr3], axis=1)  # [L, KE, H]
    wrb = np.repeat(wr[:, None, :], H, axis=1).astype(np.float32)
    nw1h = nw1[:, :H, :]
    nw1a = nw1[:, H:, :]

    flags = (
        bool(np.any(eb1)), bool(np.any(eb2)),
        bool(np.any(nb1)), bool(np.any(nb2)), bool(np.any(ln_b)),
    )

    key = (T, flags)
    if key not in _CACHE:
        _CACHE[key] = _build(T, flags)
    nc = _CACHE[key]

    ident = np.eye(H, dtype=np.float32)
    sq = (coords * coords).sum(-1).astype(np.float32)  # [N]

    shared = {
        "w1h": w1h.astype(BF), "w1c": w1c.astype(BF), "wrb": wrb,
        "w1et": w1et.astype(BF), "ew2": ew2.astype(BF),
        "nw1h": nw1h.astype(BF), "nw1a": nw1a.astype(BF), "nw2": nw2.astype(BF),
        "lngb": np.tile(ln_g, (H, 1)).astype(np.float32),
        "identb": ident.astype(BF),
    }
    if flags[0]:
        shared["eb1T"] = np.ascontiguousarray(eb1.T)
    if flags[1]:
        shared["eb2b"] = np.repeat(eb2[:, None, :], H, axis=1).astype(np.float32)
    if flags[2]:
        shared["nb1T"] = np.ascontiguousarray(nb1.T)
    if flags[3]:
        shared["nb2T"] = np.ascontiguousarray(nb2.T)
    if flags[4]:
        shared["lnbb"] = np.tile(ln_b, (H, 1)).astype(np.float32)

    in_maps = []
    for k in range(NCORES):
        pk = perm[k]
        mask = pk >= 0
        hk = np.zeros((NPCP, H), np.float32)
        hk[mask] = h[pk[mask]]
        s2k = np.zeros(NPCP, np.float32)
        s2k[mask] = sq[pk[mask]]
        et = np.zeros((KE, EPAD), np.float32)
        et[:DE] = eaS[k].T
        et[DE:] = t1[k].T
        m = {
            "hT0": np.ascontiguousarray(hk.T).astype(BF),
            "s2": np.ascontiguousarray(s2k.reshape(WINS, 128).T),
            "idxq": _wrap_idx(idxq[k]),
            "et": et.astype(BF),
            "indb": indb[k],
            "indTb": indTb[k],
        }
        m.update(shared)
        in_maps.append(m)

    trace = bool(os.environ.get("EGNN_TRACE"))
    kw = {}
    if trace:
        kw = {"trace": True, "tmpdir": os.environ.get("EGNN_TRACE_DIR") or None}
    res = run_bass_kernel_spmd(nc, in_maps, list(range(NCORES)), **kw)
    if trace:
        print(f"HW exec time: {res.exec_time_ns} ns")
    out = np.zeros((N, H), np.float32)
    for k in range(NCORES):
        pk = perm[k]
        mask = pk >= 0
        out[pk[mask]] = res.results[k]["out"][mask]
    return out


# revision 7
# speedup vs baseline: 1.9920x; 1.2976x over previous
"""EGNN (4-layer) Trainium2 kernel, 8 NeuronCores, edge-sharded.

Strategy (v2 — SWDGE-minimal):
 - Edges sorted by destination row; each core owns 2500 nodes (20 windows
   of 128). Per (core,window) uniform tile budget T.
 - Edge-MLP first matmul decomposed: concat([h_r,h_c,rad,ea]) @ W1 =
   P'win one-hot-matmul + Q'[col] gather + [W1e;wr3].T @ [ea;c_r*c_c],
   with P' = h@W1a + |c|^2*wr per-node tables (row side is window-local).
 - One-hot matrices ind [edge,node] (aggregation stationary) and
   indT [node,edge] (P-term moving operand) are HOST-built bf16 DRAM
   tensors streamed per window — no on-device is_equal chains, and the
   row-side "gather" is a matmul, eliminating one of two SWDGE gathers.
 - Q side: per-layer AllGather of per-core Q' tables to a shared DRAM
   buffer; one HBM-source transposed dma_gather per window (the only
   SWDGE user left).
 - Edge-input term: single K=35 matmul vs host-fused et=[ea;c_r*c_c].
 - m2 activations batched per 512-group; segment-sum via accumulating
   one-hot matmul producing agg^T directly.
"""

import math
import os
import sys
from contextlib import ExitStack

import numpy as np

sys.path.insert(0, "/opt/trn_rl_repo")

import ml_dtypes  # noqa: E402

BF = ml_dtypes.bfloat16

N = 20000
NCORES = 8
NPC = 2500          # nodes per core
NPCP = 2560         # padded nodes per core
WINS = 20           # node windows of 128 per core
NG = NCORES * NPCP  # global padded nodes
H = 128
DE = 32
KE = DE + 3         # fused edge-input contraction dim
L = 4
EPS = 1e-5

GATHER_PREP = os.environ.get("EGNN_PREP", "0") == "1"  # prepare_only + trigger_dma

_CACHE = {}


def _groups(T):
    """Split T tiles-of-128 into free-dim groups of <=512."""
    out = []
    left = T
    while left > 0:
        g = min(left, 4)
        out.append(g * 128)
        left -= g
    return out


def _build(T, flags):
    """Build the SPMD Bass program (same for all cores)."""
    import concourse.bacc as bacc
    import concourse.tile as tile
    from concourse import mybir

    eb1_nz, eb2_nz, nb1_nz, nb2_nz, lnb_nz = flags
    EW = T * 128
    EPAD = WINS * EW
    GRPS = _groups(T)

    f32 = mybir.dt.float32
    bf16 = mybir.dt.bfloat16
    i16 = mybir.dt.int16
    AX = mybir.AxisListType.X
    OP = mybir.AluOpType
    AF = mybir.ActivationFunctionType

    nc = bacc.Bacc(
        "TRN2",
        target_bir_lowering=False,
        debug=False,
        enable_asserts=False,
        num_devices=NCORES,
    )

    def din(name, shape, dt):
        return nc.dram_tensor(name, list(shape), dt, kind="ExternalInput").ap()

    hT0_d = din("hT0", (H, NPCP), bf16)
    s2_d = din("s2", (128, WINS), f32)
    idxq_d = din("idxq", (128, EPAD // 16), i16)
    et_d = din("et", (KE, EPAD), bf16)
    ind_d = din("indb", (128, EPAD), bf16)
    indT_d = din("indTb", (128, EPAD), bf16)
    w1h_d = din("w1h", (L, H, H), bf16)
    w1c_d = din("w1c", (L, H, H), bf16)
    wrb_d = din("wrb", (L, H, H), f32)
    w1et_d = din("w1et", (L, KE, H), bf16)
    ew2_d = din("ew2", (L, H, H), bf16)
    nw1h_d = din("nw1h", (L, H, H), bf16)
    nw1a_d = din("nw1a", (L, H, H), bf16)
    nw2_d = din("nw2", (L, H, H), bf16)
    eb1_d = din("eb1T", (H, L), f32) if eb1_nz else None
    nb1_d = din("nb1T", (H, L), f32) if nb1_nz else None
    nb2_d = din("nb2T", (H, L), f32) if nb2_nz else None
    eb2b_d = din("eb2b", (L, H, H), f32) if eb2_nz else None
    lng_d = din("lngb", (H, H), f32)
    lnb_d = din("lnbb", (H, H), f32) if lnb_nz else None
    idb_d = din("identb", (H, H), bf16)

    out_d = nc.dram_tensor("out", [NPCP, H], f32, kind="ExternalOutput").ap()

    with ExitStack() as ctx:
        tc = ctx.enter_context(tile.TileContext(nc))
        const = ctx.enter_context(tc.tile_pool(name="const", bufs=1))
        resid = ctx.enter_context(tc.tile_pool(name="resid", bufs=1))
        dram = ctx.enter_context(tc.tile_pool(name="dram", bufs=1, space="DRAM"))
        gpool = ctx.enter_context(tc.tile_pool(name="gpool", bufs=4))
        wpool = ctx.enter_context(tc.tile_pool(name="wpool", bufs=3))
        work = ctx.enter_context(tc.tile_pool(name="work", bufs=3))
        ps512 = ctx.enter_context(tc.tile_pool(name="ps512", bufs=3, space="PSUM"))
        psm = ctx.enter_context(tc.tile_pool(name="psm", bufs=2, space="PSUM"))
        psagg = ctx.enter_context(tc.tile_pool(name="psagg", bufs=2, space="PSUM"))

        sync = nc.sync

        # ---------- constants ----------
        idb_sb = const.tile([H, H], bf16)
        sync.dma_start(out=idb_sb[:], in_=idb_d[:])
        lng_sb = const.tile([H, H], f32)
        sync.dma_start(out=lng_sb[:], in_=lng_d[:])
        lnb_sb = None
        if lnb_nz:
            lnb_sb = const.tile([H, H], f32)
            sync.dma_start(out=lnb_sb[:], in_=lnb_d[:])
        eb1_sb = None
        if eb1_nz:
            eb1_sb = const.tile([H, L], f32)
            sync.dma_start(out=eb1_sb[:], in_=eb1_d[:])
        nb1_sb = None
        if nb1_nz:
            nb1_sb = const.tile([H, L], f32)
            sync.dma_start(out=nb1_sb[:], in_=nb1_d[:])
        nb2_sb = None
        if nb2_nz:
            nb2_sb = const.tile([H, L], f32)
            sync.dma_start(out=nb2_sb[:], in_=nb2_d[:])

        def load_w(name, d, p, dt):
            t = const.tile([p, L, H], dt, name=name)
            sync.dma_start(out=t[:], in_=d.rearrange("l k f -> k l f"))
            return t

        w1h_sb = load_w("w1h_sb", w1h_d, H, bf16)
        w1c_sb = load_w("w1c_sb", w1c_d, H, bf16)
        wrb_sb = load_w("wrb_sb", wrb_d, H, f32)
        w1et_sb = load_w("w1et_sb", w1et_d, KE, bf16)
        ew2_sb = load_w("ew2_sb", ew2_d, H, bf16)
        nw1h_sb = load_w("nw1h_sb", nw1h_d, H, bf16)
        nw1a_sb = load_w("nw1a_sb", nw1a_d, H, bf16)
        nw2_sb = load_w("nw2_sb", nw2_d, H, bf16)
        eb2b_sb = load_w("eb2b_sb", eb2b_d, H, f32) if eb2_nz else None

        # ---------- resident state ----------
        hT = resid.tile([H, NPCP], bf16)
        sync.dma_start(out=hT[:], in_=hT0_d[:])
        zTa = resid.tile([H, NPCP], bf16)
        s2_sb = resid.tile([128, WINS], f32)
        sync.dma_start(out=s2_sb[:], in_=s2_d[:])
        p_sb = resid.tile([128, WINS * H], bf16)
        idxq_sb = resid.tile([128, EPAD // 16], i16)
        sync.dma_start(out=idxq_sb[:], in_=idxq_d[:])

        qown_dram = dram.tile([NPCP, H], bf16)
        gsem = nc.alloc_semaphore("gq_dma") if GATHER_PREP else None

        # ---------- layers ----------
        for l in range(L):
            # node tables Q' then P' (Q first so the AllGather launches early)
            for j in range(WINS):
                jsl = slice(j * H, (j + 1) * H)
                tfold = work.tile([128, H], f32, tag="tfold")
                nc.vector.tensor_scalar_mul(
                    tfold[:], wrb_sb[:, l, :], s2_sb[:, j : j + 1]
                )
                psq = psm.tile([H, H], f32, tag="pm")
                nc.tensor.matmul(
                    psq[:], hT[:, jsl], w1c_sb[:, l, :], start=True, stop=True
                )
                qt = work.tile([128, H], bf16, tag="qt")
                nc.vector.tensor_tensor(qt[:], psq[:], tfold[:], OP.add)
                sync.dma_start(out=qown_dram[jsl, :], in_=qt[:])
                psp = psm.tile([H, H], f32, tag="pm")
                nc.tensor.matmul(
                    psp[:], hT[:, jsl], w1h_sb[:, l, :], start=True, stop=True
                )
                nc.vector.tensor_tensor(p_sb[:, jsl], psp[:], tfold[:], OP.add)

            qfull_dram = dram.tile(
                [NG, H], bf16, addr_space="Shared", name=f"qfull_{l}"
            )
            nc.gpsimd.collective_compute(
                "AllGather",
                mybir.AluOpType.bypass,
                replica_groups=[list(range(NCORES))],
                ins=[qown_dram.opt()],
                outs=[qfull_dram.opt()],
            )

            # edge pass
            for w in range(WINS):
                wsl = slice(w * EW, (w + 1) * EW)
                isl = slice(w * (EW // 16), (w + 1) * (EW // 16))
                psl = slice(w * H, (w + 1) * H)
                gq = gpool.tile([128, 1, EW], bf16, tag="gq")
                if GATHER_PREP:
                    nc.gpsimd.dma_gather(
                        gq[:], qfull_dram[:], idxq_sb[:, isl], EW, EW, H,
                        transpose=True, single_packet=False,
                        prepare_only=True, sem=gsem,
                    )
                    nc.gpsimd.trigger_dma(count=None)
                else:
                    nc.gpsimd.dma_gather(
                        gq[:], qfull_dram[:], idxq_sb[:, isl], EW, EW, H,
                        transpose=True, single_packet=False,
                    )
                etw = wpool.tile([KE, EW], bf16, tag="etw")
                sync.dma_start(out=etw[:], in_=et_d[:, wsl])
                indw = wpool.tile([128, EW], bf16, tag="indw")
                sync.dma_start(out=indw[:], in_=ind_d[:, wsl])
                indTw = wpool.tile([128, EW], bf16, tag="indTw")
                sync.dma_start(out=indTw[:], in_=indT_d[:, wsl])

                pagg = psagg.tile([H, H], f32, tag="pagg")
                gtile = 0
                off = 0
                for gsz in GRPS:
                    gsl = slice(off, off + gsz)
                    ps1 = ps512.tile([H, 512], f32, tag="big")
                    nc.tensor.matmul(
                        ps1[:, :gsz], w1et_sb[:, l, :], etw[:, gsl],
                        start=True, stop=False,
                    )
                    nc.tensor.matmul(
                        ps1[:, :gsz], p_sb[:, psl], indTw[:, gsl],
                        start=False, stop=True,
                    )
                    pre1 = work.tile([H, 512], f32, tag="pre1")
                    nc.vector.tensor_tensor(
                        pre1[:, :gsz], ps1[:, :gsz], gq[:, 0, gsl], OP.add
                    )
                    m1t = work.tile([H, 512], bf16, tag="m1t")
                    bias1 = eb1_sb[:, l : l + 1] if eb1_nz else 0.0
                    nc.scalar.activation(m1t[:, :gsz], pre1[:, :gsz], AF.Silu, bias=bias1)

                    pm2 = ps512.tile([H, 512], f32, tag="big")
                    for t in range(gsz // 128):
                        tsl = slice(t * 128, (t + 1) * 128)
                        nc.tensor.matmul(
                            pm2[:, tsl], m1t[:, tsl], ew2_sb[:, l, :],
                            start=True, stop=True,
                        )
                    m2s = work.tile([H, 512], bf16, tag="m2s")
                    if eb2_nz:
                        tm2 = work.tile([H, 512], f32, tag="tm2")
                        eb2g = eb2b_sb[:, l, :]
                        for t in range(gsz // 128):
                            tsl = slice(t * 128, (t + 1) * 128)
                            nc.vector.tensor_tensor(
                                tm2[:, tsl], pm2[:, tsl], eb2g, OP.add
                            )
                        nc.scalar.activation(m2s[:, :gsz], tm2[:, :gsz], AF.Silu)
                    else:
                        nc.scalar.activation(m2s[:, :gsz], pm2[:, :gsz], AF.Silu)
                    for t in range(gsz // 128):
                        tsl = slice(t * 128, (t + 1) * 128)
                        nc.tensor.matmul(
                            pagg[:], m2s[:, tsl], indw[:, gtile * 128 : gtile * 128 + 128],
                            start=(gtile == 0), stop=(gtile == T - 1),
                        )
                        gtile += 1
                    off += gsz
                nc.vector.tensor_copy(zTa[:, psl], pagg[:])

            # node MLP (+ residual)
            for g5 in range(NPCP // 512):
                sl = slice(g5 * 512, (g5 + 1) * 512)
                psu = ps512.tile([H, 512], f32, tag="big")
                nc.tensor.matmul(
                    psu[:], nw1h_sb[:, l, :], hT[:, sl], start=True, stop=False
                )
                nc.tensor.matmul(
                    psu[:], nw1a_sb[:, l, :], zTa[:, sl], start=False, stop=True
                )
                u = work.tile([H, 512], bf16, tag="u")
                biasn = nb1_sb[:, l : l + 1] if nb1_nz else 0.0
                nc.scalar.activation(u[:], psu[:], AF.Silu, bias=biasn)
                pso = ps512.tile([H, 512], f32, tag="big")
                nc.tensor.matmul(pso[:], nw2_sb[:, l, :], u[:], start=True, stop=True)
                if l == 0:
                    if nb2_nz:
                        nc.vector.tensor_scalar_add(
                            hT[:, sl], pso[:], nb2_sb[:, l : l + 1]
                        )
                    else:
                        nc.vector.tensor_copy(hT[:, sl], pso[:])
                else:
                    if nb2_nz:
                        nc.vector.scalar_tensor_tensor(
                            hT[:, sl], pso[:], nb2_sb[:, l : l + 1], hT[:, sl],
                            OP.add, OP.add,
                        )
                    else:
                        nc.vector.tensor_tensor(hT[:, sl], pso[:], hT[:, sl], OP.add)

        # ---------- LayerNorm + output ----------
        inv = 1.0 / H
        for j in range(WINS):
            jsl = slice(j * H, (j + 1) * H)
            pst = psm.tile([H, H], bf16, tag="pm")
            nc.tensor.transpose(pst[:], hT[:, jsl], idb_sb[:])
            hn = work.tile([H, H], f32, tag="hn")
            nc.vector.tensor_copy(hn[:], pst[:])
            mu = work.tile([H, 1], f32, tag="mu")
            nc.vector.reduce_sum(mu[:], hn[:], AX)
            nc.vector.tensor_scalar_mul(mu[:], mu[:], inv)
            xc = work.tile([H, H], f32, tag="xc")
            nc.vector.tensor_scalar_sub(xc[:], hn[:], mu[:])
            sq = work.tile([H, H], f32, tag="sq")
            nc.vector.tensor_mul(sq[:], xc[:], xc[:])
            var = work.tile([H, 1], f32, tag="var")
            nc.vector.reduce_sum(var[:], sq[:], AX)
            sd = work.tile([H, 1], f32, tag="sd")
            nc.vector.tensor_scalar(sd[:], var[:], inv, EPS, OP.mult, OP.add)
            nc.scalar.activation(sd[:], sd[:], mybir.ActivationFunctionType.Sqrt)
            rstd = work.tile([H, 1], f32, tag="rstd")
            nc.vector.reciprocal(rstd[:], sd[:])
            on = work.tile([H, H], f32, tag="on")
            nc.vector.tensor_scalar_mul(on[:], xc[:], rstd[:])
            nc.vector.tensor_mul(on[:], on[:], lng_sb[:])
            if lnb_nz:
                nc.vector.tensor_add(on[:], on[:], lnb_sb[:])
            sync.dma_start(out=out_d[jsl, :], in_=on[:])

    nc.compile()
    return nc


def _wrap_idx(v):
    """idx i -> [i%16 partition, i//16 free], replicated to 128 partitions."""
    n = v.shape[0]
    t = v.reshape(n // 16, 16).T.astype(np.int16)
    return np.tile(t, (8, 1))


def _prep(edges, edge_attr, coords):
    """Edge sorting + balanced node->window binning + per-edge layouts.

    Returns dict with T/EW/EPAD, per-core edge arrays (valid, eaS, t1,
    rowrel, idxq) and the node permutation perm [NC, NPCP] (node id per
    (window,slot); -1 for pads).
    """
    E = edges.shape[1]
    row, col = edges[0], edges[1]

    # balanced binning: per core, assign nodes to 20 windows (<=128 each)
    # minimizing the max per-window edge count (LPT greedy on degree)
    deg = np.bincount(row, minlength=N)
    win_of = np.zeros(N, np.int32)
    binpos = np.zeros(N, np.int32)   # w*128 + slot within core
    perm = np.full((NCORES, NPCP), -1, np.int64)
    for k in range(NCORES):
        dk = deg[k * NPC : (k + 1) * NPC]
        orderk = np.argsort(-dk, kind="stable")
        loads = np.zeros(WINS, np.int64)
        slots = np.zeros(WINS, np.int64)
        big = 1 << 60
        for nl in orderk:
            w = int(np.argmin(np.where(slots < 128, loads, big)))
            g = k * NPC + nl
            win_of[g] = w
            binpos[g] = w * 128 + slots[w]
            perm[k, w * 128 + slots[w]] = g
            slots[w] += 1
            loads[w] += dk[nl]

    owner = row // NPC
    gw = owner * WINS + win_of[row]
    order = np.argsort(gw, kind="stable")
    counts = np.bincount(gw, minlength=NCORES * WINS)
    T = int(math.ceil(counts.max() / 128))
    EW = T * 128
    EPAD = WINS * EW

    gws = gw[order]
    starts = np.zeros(NCORES * WINS, np.int64)
    starts[1:] = np.cumsum(counts)[:-1]
    pos = np.arange(E) - starts[gws]
    slot = (gws % WINS) * EW + pos
    core = gws // WINS

    rowS = np.zeros((NCORES, EPAD), np.int64)
    colS = np.zeros((NCORES, EPAD), np.int64)
    valid = np.zeros((NCORES, EPAD), bool)
    eaS = np.zeros((NCORES, EPAD, DE), np.float32)
    rowS[core, slot] = row[order]
    colS[core, slot] = col[order]
    valid[core, slot] = True
    eaS[core, slot] = edge_attr[order]

    rowrel = np.where(valid, binpos[rowS] % 128, -1).astype(np.int32)
    gcol = (colS // NPC) * NPCP + binpos[colS]
    idxq = np.where(valid, gcol, 0).astype(np.int64)

    crS = np.where(valid[..., None], coords[rowS], 0.0).astype(np.float32)
    ccS = np.where(valid[..., None], coords[colS], 0.0).astype(np.float32)
    t1 = crS * ccS  # [NC, EPAD, 3]

    return dict(T=T, EW=EW, EPAD=EPAD, valid=valid, eaS=eaS, t1=t1,
                rowrel=rowrel, idxq=idxq, perm=perm)


def kernel(**inputs):
    from concourse.bass_utils import run_bass_kernel_spmd

    h = np.asarray(inputs["h"], np.float32)
    coords = np.asarray(inputs["coords"], np.float32)
    edge_attr = np.asarray(inputs["edge_attr"], np.float32)
    edges = np.asarray(inputs["edges"]).astype(np.int64)
    ew1 = np.asarray(inputs["edge_w1"], np.float32)
    eb1 = np.asarray(inputs["edge_b1"], np.float32)
    ew2 = np.asarray(inputs["edge_w2"], np.float32)
    eb2 = np.asarray(inputs["edge_b2"], np.float32)
    nw1 = np.asarray(inputs["node_w1"], np.float32)
    nb1 = np.asarray(inputs["node_b1"], np.float32)
    nw2 = np.asarray(inputs["node_w2"], np.float32)
    nb2 = np.asarray(inputs["node_b2"], np.float32)
    ln_g = np.asarray(inputs["ln_g"], np.float32)
    ln_b = np.asarray(inputs["ln_b"], np.float32)

    P = _prep(edges, edge_attr, coords)
    T, EW, EPAD = P["T"], P["EW"], P["EPAD"]
    valid, eaS, t1 = P["valid"], P["eaS"], P["t1"]
    rowrel, idxq, perm = P["rowrel"], P["idxq"], P["perm"]

    # host-built one-hots (bf16)
    pp = np.arange(128)
    rr3 = rowrel.reshape(NCORES, EPAD // 128, 128)
    # ind[k, p, gti*128+f] = (rowrel[k, gti*128+p] == f)  [agg stationary]
    indb = (
        (rr3[:, :, :, None] == pp[None, None, None, :])
        .transpose(0, 2, 1, 3)
        .reshape(NCORES, 128, EPAD)
        .astype(BF)
    )
    # indT[k, p, e] = (rowrel[k, e] == p)                  [P-term moving]
    indTb = (
        (rr3[:, :, :, None] == pp[None, None, None, :])
        .transpose(0, 3, 1, 2)
        .reshape(NCORES, 128, EPAD)
        .astype(BF)
    )

    # ---- weights ----
    w1h = ew1[:, 0:H, :]
    w1c = ew1[:, H : 2 * H, :]
    wr = ew1[:, 2 * H, :]          # [L, H]
    w1e = ew1[:, 2 * H + 1 :, :]   # [L, DE, H]
    wr3 = np.repeat((-2.0 * wr)[:, None, :], 3, axis=1)
    w1et = np.concatenate([w1e, wr3], axis=1)  # [L, KE, H]
    wrb = np.repeat(wr[:, None, :], H, axis=1).astype(np.float32)
    nw1h = nw1[:, :H, :]
    nw1a = nw1[:, H:, :]

    flags = (
        bool(np.any(eb1)), bool(np.any(eb2)),
        bool(np.any(nb1)), bool(np.any(nb2)), bool(np.any(ln_b)),
    )

    key = (T, flags)
    if key not in _CACHE:
        _CACHE[key] = _build(T, flags)
    nc = _CACHE[key]

    ident = np.eye(H, dtype=np.float32)
    sq = (coords * coords).sum(-1).astype(np.float32)  # [N]

    shared = {
        "w1h": w1h.astype(BF), "w1c": w1c.astype(BF), "wrb": wrb,
        "w1et": w1et.astype(BF), "ew2": ew2.astype(BF),
        "nw1h": nw1h.astype(BF), "nw1a": nw1a.astype(BF), "nw2": nw2.astype(BF),
        "lngb": np.tile(ln_g, (H, 1)).astype(np.float32),
        "identb": ident.astype(BF),
    }
    if flags[0]:
        shared["eb1T"] = np.ascontiguousarray(eb1.T)
    if flags[1]:
        shared["eb2b"] = np.repeat(eb2[:, None, :], H, axis=1).astype(np.float32)
    if flags[2]:
        shared["nb1T"] = np.ascontiguousarray(nb1.T)
    if flags[3]:
        shared["nb2T"] = np.ascontiguousarray(nb2.T)
    if flags[4]:
        shared["lnbb"] = np.tile(ln_b, (H, 1)).astype(np.float32)

    in_maps = []
    for k in range(NCORES):
        pk = perm[k]
        mask = pk >= 0
        hk = np.zeros((NPCP, H), np.float32)
        hk[mask] = h[pk[mask]]
        s2k = np.zeros(NPCP, np.float32)
        s2k[mask] = sq[pk[mask]]
        et = np.zeros((KE, EPAD), np.float32)
        et[:DE] = eaS[k].T
        et[DE:] = t1[k].T
        m = {
            "hT0": np.ascontiguousarray(hk.T).astype(BF),
            "s2": np.ascontiguousarray(s2k.reshape(WINS, 128).T),
            "idxq": _wrap_idx(idxq[k]),
            "et": et.astype(BF),
            "indb": indb[k],
            "indTb": indTb[k],
        }
        m.update(shared)
        in_maps.append(m)

    trace = bool(os.environ.get("EGNN_TRACE"))
    kw = {}
    if trace:
        kw = {"trace": True, "tmpdir": os.environ.get("EGNN_TRACE_DIR") or None}
    res = run_bass_kernel_spmd(nc, in_maps, list(range(NCORES)), **kw)
    if trace:
        print(f"HW exec time: {res.exec_time_ns} ns")
    out = np.zeros((N, H), np.float32)
    for k in range(NCORES):
        pk = perm[k]
        mask = pk >= 0
        out[pk[mask]] = res.results[k]["out"][mask]
    return out
